# revision 1
# baseline (speedup 1.0000x reference)
"""2D Haar DWT (periodization) on Trainium2, data-parallel over 8 NeuronCores.

Input  x: [8, 32, 512, 512] f32  (batch, channel, H, W)
Output (LL, LH, HL, HH), each [8, 32, 256, 256] f32.

Sharding: batch -> 8 cores (one batch element per core, fully local).

Per-core layout: the [32, 512, 512] slice is viewed as 16384 contiguous
rows of 512 floats. Each SBUF partition holds RPP consecutive rows
(RPP/2 H-pairs), so every DMA is a single fully-contiguous block:
  - input tile  [128, RPP*512] f32 (2 MiB for RPP=8)
  - output tile [128, RPP/2*256] per subband (512 KiB for RPP=8)
Butterfly on DVE (all tensor_tensor, fp32 1x):
  stage 1 (H pairs, within-partition contiguous slices):
      S = E + O ; D = E - O
  stage 2 (W pairs, stride-2 views):
      LL = S_e + S_o ; HL = S_e - S_o ; LH = D_e + D_o ; HH = D_e - D_o
The single 0.5 of the separable transform is folded into one in-place
ScalarE pass per output tile right before its store.

The last full tile is split into 4 small subtiles to shorten the
end-of-kernel compute tail behind the final input DMA.
"""

import sys

import numpy as np

if "/opt/trn_rl_repo" not in sys.path:
    sys.path.insert(0, "/opt/trn_rl_repo")

B, C, H, W = 8, 32, 512, 512
ROWS = C * H              # 16384 flat rows per core
RPP = 8                   # input rows per partition (must be even)
TILE_ROWS = 128 * RPP     # 1024
OROWS = ROWS // 2         # 8192 output rows per subband per core
N_CORES = 8

# (row0, nrows) plan: full tiles, then the last tile tapered 4x smaller.
TAPER = 4
PLAN = [(i * TILE_ROWS, TILE_ROWS) for i in range(ROWS // TILE_ROWS - 1)]
PLAN += [
    ((ROWS - TILE_ROWS) + k * (TILE_ROWS // TAPER), TILE_ROWS // TAPER)
    for k in range(TAPER)
]

SUBBANDS = ("ll", "lh", "hl", "hh")

_cache = {}


def _build_program():
    from concourse import bacc, mybir
    from concourse.tile import TileContext

    f32 = mybir.dt.float32
    add = mybir.AluOpType.add
    sub = mybir.AluOpType.subtract

    # Bacc (not raw Bass): its compile() runs generate_event_semaphores(),
    # which splits multi-wait instructions down to the TRN2 limit of one
    # sync wait per instruction — walrus codegen rejects the raw form.
    nc = bacc.Bacc()
    x = nc.dram_tensor("x", [ROWS, W], f32, kind="ExternalInput")
    out = {
        n: nc.dram_tensor(n, [OROWS, W // 2], f32, kind="ExternalOutput")
        for n in SUBBANDS
    }

    with TileContext(nc) as tc, tc.tile_pool(name="p", bufs=3) as pool:
        for r0, nrows in PLAN:
            rpp = nrows // 128        # rows per partition this tile
            jp = rpp // 2             # H-pairs per partition
            # 2D contiguous DMAs everywhere: DRAM side is a plain row
            # slice, SBUF side a flat [128, free] tile (flat iteration
            # orders match elementwise).
            tin = pool.tile([128, rpp * W], f32, tag="tin",
                            padded_shape=[128, RPP * W])
            nc.sync.dma_start(tin[:], x[r0 : r0 + nrows, :])

            t4 = tin.rearrange("p (j o w) -> p j o w", j=jp, o=2)
            e = t4[:, :, 0, :]    # even H rows  [128, jp, 512]
            o = t4[:, :, 1, :]    # odd H rows   [128, jp, 512]

            s = pool.tile([128, jp * W], f32, tag="s",
                          padded_shape=[128, (RPP // 2) * W])
            d = pool.tile([128, jp * W], f32, tag="d",
                          padded_shape=[128, (RPP // 2) * W])
            s3 = s.rearrange("p (j w) -> p j w", j=jp)
            d3 = d.rearrange("p (j w) -> p j w", j=jp)
            nc.vector.tensor_add(out=s3, in0=e, in1=o)
            nc.vector.tensor_sub(out=d3, in0=e, in1=o)

            s4 = s.rearrange("p (j k o) -> p j k o", j=jp, o=2)
            d4 = d.rearrange("p (j k o) -> p j k o", j=jp, o=2)
            se, so = s4[:, :, :, 0], s4[:, :, :, 1]
            de, do = d4[:, :, :, 0], d4[:, :, :, 1]

            ob = {n: pool.tile([128, jp * (W // 2)], f32, tag=n, name=n,
                               padded_shape=[128, (RPP // 2) * (W // 2)])
                  for n in SUBBANDS}
            o3 = {n: ob[n].rearrange("p (j w) -> p j w", j=jp)
                  for n in SUBBANDS}
            nc.vector.tensor_tensor(out=o3["ll"], in0=se, in1=so, op=add)
            nc.vector.tensor_tensor(out=o3["hl"], in0=se, in1=so, op=sub)
            nc.vector.tensor_tensor(out=o3["lh"], in0=de, in1=do, op=add)
            nc.vector.tensor_tensor(out=o3["hh"], in0=de, in1=do, op=sub)

            orow = r0 // 2
            for n in SUBBANDS:
                nc.scalar.mul(ob[n][:], ob[n][:], 0.5)
                nc.sync.dma_start(
                    out[n][orow : orow + nrows // 2, :], ob[n][:]
                )

    nc.finalize()
    return nc


def _run(x, trace=False):
    from concourse.bass_utils import run_bass_kernel_spmd

    if "nc" not in _cache:
        _cache["nc"] = _build_program()
    nc = _cache["nc"]

    x = np.ascontiguousarray(np.asarray(x), dtype=np.float32)
    in_maps = [{"x": x[i].reshape(ROWS, W)} for i in range(N_CORES)]
    res = run_bass_kernel_spmd(nc, in_maps, core_ids=list(range(N_CORES)), trace=trace)
    _cache["last_results"] = res

    outs = []
    for n in ("ll", "lh", "hl", "hh"):
        outs.append(
            np.stack([res.results[i][n].reshape(C, H // 2, W // 2)
                      for i in range(N_CORES)])
        )
    return tuple(outs)


def kernel(x):
    return _run(x, trace=False)



# revision 2
# speedup vs baseline: 1.0832x; 1.0832x over previous
"""2D Haar DWT (periodization) on Trainium2, data-parallel over 8 NeuronCores.

v4 = v2c + DMA-trigger distribution + front taper:
  - Sync sequencer triggers ONLY input DMAs -> input prefetch is never
    blocked behind output-store semaphore waits (v2c showed Sync spending
    ~10 us/tile blocked, serializing the pipeline).
  - ScalarE triggers each subband store immediately after its own x0.5
    pass (same-engine program order, no cross-engine wait).
  - First and last tiles split 4x smaller to shorten pipeline ramp/drain.

Engine load per full tile: DVE 6.7us (stage1 + ll/hl), GpSimd 5.7us
(lh/hh), ScalarE ~7us (4 muls + 4 store triggers), DMA queues 9.8us.

Input  x: [8, 32, 512, 512] f32 -> batch -> 8 cores.
Output (LL, LH, HL, HH), each [8, 32, 256, 256] f32.
"""

import sys

import numpy as np

if "/opt/trn_rl_repo" not in sys.path:
    sys.path.insert(0, "/opt/trn_rl_repo")

B, C, H, W = 8, 32, 512, 512
ROWS = C * H              # 16384 flat rows per core
RPP = 8                   # input rows per partition (must be even)
TILE_ROWS = 128 * RPP     # 1024
OROWS = ROWS // 2         # 8192 output rows per subband per core
N_CORES = 8

TAPER = 4
QT = TILE_ROWS // TAPER   # 256
PLAN = [(k * QT, QT) for k in range(TAPER)]                   # front taper
PLAN += [(i * TILE_ROWS, TILE_ROWS)
         for i in range(1, ROWS // TILE_ROWS - 1)]            # full tiles
PLAN += [((ROWS - TILE_ROWS) + k * QT, QT) for k in range(TAPER)]  # tail taper

SUBBANDS = ("ll", "lh", "hl", "hh")

_cache = {}


def _build_program():
    from concourse import bacc, mybir
    from concourse.tile import TileContext

    f32 = mybir.dt.float32
    add = mybir.AluOpType.add
    sub = mybir.AluOpType.subtract

    nc = bacc.Bacc()
    x = nc.dram_tensor("x", [ROWS, W], f32, kind="ExternalInput")
    out = {
        n: nc.dram_tensor(n, [OROWS, W // 2], f32, kind="ExternalOutput")
        for n in SUBBANDS
    }

    with TileContext(nc) as tc, tc.tile_pool(name="p", bufs=4) as pool:
        for r0, nrows in PLAN:
            rpp = nrows // 128        # rows per partition this tile
            jp = rpp // 2             # H-pairs per partition
            tin = pool.tile([128, rpp * W], f32, tag="tin",
                            padded_shape=[128, RPP * W])
            nc.sync.dma_start(tin[:], x[r0 : r0 + nrows, :])

            t4 = tin.rearrange("p (j o w) -> p j o w", j=jp, o=2)
            e = t4[:, :, 0, :]    # even H rows  [128, jp, 512]
            o = t4[:, :, 1, :]    # odd H rows   [128, jp, 512]

            s = pool.tile([128, jp * W], f32, tag="s",
                          padded_shape=[128, (RPP // 2) * W])
            d = pool.tile([128, jp * W], f32, tag="d",
                          padded_shape=[128, (RPP // 2) * W])
            s3 = s.rearrange("p (j w) -> p j w", j=jp)
            d3 = d.rearrange("p (j w) -> p j w", j=jp)
            nc.vector.tensor_add(out=s3, in0=e, in1=o)
            nc.vector.tensor_sub(out=d3, in0=e, in1=o)

            s4 = s.rearrange("p (j k o) -> p j k o", j=jp, o=2)
            d4 = d.rearrange("p (j k o) -> p j k o", j=jp, o=2)
            se, so = s4[:, :, :, 0], s4[:, :, :, 1]
            de, do = d4[:, :, :, 0], d4[:, :, :, 1]

            ob = {n: pool.tile([128, jp * (W // 2)], f32, tag=n, name=n,
                               padded_shape=[128, (RPP // 2) * (W // 2)])
                  for n in SUBBANDS}
            o3 = {n: ob[n].rearrange("p (j w) -> p j w", j=jp)
                  for n in SUBBANDS}
            nc.vector.tensor_tensor(out=o3["ll"], in0=se, in1=so, op=add)
            nc.gpsimd.tensor_tensor(out=o3["lh"], in0=de, in1=do, op=add)
            nc.vector.tensor_tensor(out=o3["hl"], in0=se, in1=so, op=sub)
            nc.gpsimd.tensor_tensor(out=o3["hh"], in0=de, in1=do, op=sub)

            orow = r0 // 2
            for n in SUBBANDS:
                nc.scalar.mul(ob[n][:], ob[n][:], 0.5)
                nc.scalar.dma_start(
                    out[n][orow : orow + nrows // 2, :], ob[n][:]
                )

    nc.finalize()
    return nc


def _run(x, trace=False):
    from concourse.bass_utils import run_bass_kernel_spmd

    if "nc" not in _cache:
        _cache["nc"] = _build_program()
    nc = _cache["nc"]

    x = np.ascontiguousarray(np.asarray(x), dtype=np.float32)
    in_maps = [{"x": x[i].reshape(ROWS, W)} for i in range(N_CORES)]
    res = run_bass_kernel_spmd(nc, in_maps, core_ids=list(range(N_CORES)), trace=trace)
    _cache["last_results"] = res

    outs = []
    for n in ("ll", "lh", "hl", "hh"):
        outs.append(
            np.stack([res.results[i][n].reshape(C, H // 2, W // 2)
                      for i in range(N_CORES)])
        )
    return tuple(outs)


def kernel(x):
    return _run(x, trace=False)


# revision 3
# speedup vs baseline: 1.2578x; 1.1611x over previous
"""2D Haar DWT (periodization) on Trainium2, data-parallel over 8 NeuronCores.

Design (memory-bound problem; per-core floor is the 16 SDMA engines at
~27 GB/s each on 4-16 KiB descriptors):
  - batch -> 8 cores; per core the [32,512,512] plane is 16384 rows of
    512 f32, tiled 8 rows/partition (2 MiB input tiles, all DMA lines
    fully contiguous and >= 4 KiB).
  - stage 1 (H butterfly) on DVE: contiguous tensor_add/sub, full rate.
  - stage 2 (W butterfly, inherently stride-2) split across engines so
    no engine exceeds the 9.8 us/tile DMA-queue work: ll/hl on DVE,
    lh/hh on GpSimd (fp32 stride-2 = 8 B, below GpSimd's stride cliff).
  - the single x0.5 of the separable transform on ScalarE per subband.
  - Sync sequencer triggers ONLY input DMAs -> input prefetch never
    blocks behind output-store semaphore waits; ScalarE triggers each
    subband store right after its own x0.5 pass (same-engine program
    order, no cross-engine wait).  (gpsimd.dma_start is SWDGE - slow
    software path - never use it.)
  - first and last tiles split 4x smaller to shorten pipeline ramp/drain.

Engine load per full tile: DVE ~6.7-9us (stage1 + ll/hl, POOL-port
contention with GpSimd), GpSimd ~5.7us (lh/hh), ScalarE ~7us (4 muls +
4 store triggers), DMA queues 9.8us -> steady state is DMA-bound with
SDMA engines ~97% busy; ~190-194 us vs ~187 us descriptor-rate floor
(212-219 us baseline).

Input  x: [8, 32, 512, 512] f32 -> batch -> 8 cores.
Output (LL, LH, HL, HH), each [8, 32, 256, 256] f32.
"""

import sys

import numpy as np

if "/opt/trn_rl_repo" not in sys.path:
    sys.path.insert(0, "/opt/trn_rl_repo")

B, C, H, W = 8, 32, 512, 512
ROWS = C * H              # 16384 flat rows per core
RPP = 8                   # input rows per partition (must be even)
TILE_ROWS = 128 * RPP     # 1024
OROWS = ROWS // 2         # 8192 output rows per subband per core
N_CORES = 8

TAPER = 4
QT = TILE_ROWS // TAPER   # 256
PLAN = [(k * QT, QT) for k in range(TAPER)]                   # front taper
PLAN += [(i * TILE_ROWS, TILE_ROWS)
         for i in range(1, ROWS // TILE_ROWS - 1)]            # full tiles
PLAN += [((ROWS - TILE_ROWS) + k * QT, QT) for k in range(TAPER)]  # tail taper

SUBBANDS = ("ll", "lh", "hl", "hh")

_cache = {}


def _build_program():
    from concourse import bacc, mybir
    from concourse.tile import TileContext

    f32 = mybir.dt.float32
    add = mybir.AluOpType.add
    sub = mybir.AluOpType.subtract

    nc = bacc.Bacc()
    x = nc.dram_tensor("x", [ROWS, W], f32, kind="ExternalInput")
    out = {
        n: nc.dram_tensor(n, [OROWS, W // 2], f32, kind="ExternalOutput")
        for n in SUBBANDS
    }

    with TileContext(nc) as tc, tc.tile_pool(name="p", bufs=4) as pool:
        for r0, nrows in PLAN:
            rpp = nrows // 128        # rows per partition this tile
            jp = rpp // 2             # H-pairs per partition
            tin = pool.tile([128, rpp * W], f32, tag="tin",
                            padded_shape=[128, RPP * W])
            nc.sync.dma_start(tin[:], x[r0 : r0 + nrows, :])

            t4 = tin.rearrange("p (j o w) -> p j o w", j=jp, o=2)
            e = t4[:, :, 0, :]    # even H rows  [128, jp, 512]
            o = t4[:, :, 1, :]    # odd H rows   [128, jp, 512]

            s = pool.tile([128, jp * W], f32, tag="s",
                          padded_shape=[128, (RPP // 2) * W])
            d = pool.tile([128, jp * W], f32, tag="d",
                          padded_shape=[128, (RPP // 2) * W])
            s3 = s.rearrange("p (j w) -> p j w", j=jp)
            d3 = d.rearrange("p (j w) -> p j w", j=jp)
            nc.vector.tensor_add(out=s3, in0=e, in1=o)
            nc.vector.tensor_sub(out=d3, in0=e, in1=o)

            s4 = s.rearrange("p (j k o) -> p j k o", j=jp, o=2)
            d4 = d.rearrange("p (j k o) -> p j k o", j=jp, o=2)
            se, so = s4[:, :, :, 0], s4[:, :, :, 1]
            de, do = d4[:, :, :, 0], d4[:, :, :, 1]

            ob = {n: pool.tile([128, jp * (W // 2)], f32, tag=n, name=n,
                               padded_shape=[128, (RPP // 2) * (W // 2)])
                  for n in SUBBANDS}
            o3 = {n: ob[n].rearrange("p (j w) -> p j w", j=jp)
                  for n in SUBBANDS}
            nc.vector.tensor_tensor(out=o3["ll"], in0=se, in1=so, op=add)
            nc.gpsimd.tensor_tensor(out=o3["lh"], in0=de, in1=do, op=add)
            nc.vector.tensor_tensor(out=o3["hl"], in0=se, in1=so, op=sub)
            nc.gpsimd.tensor_tensor(out=o3["hh"], in0=de, in1=do, op=sub)

            orow = r0 // 2
            for n in SUBBANDS:
                nc.scalar.mul(ob[n][:], ob[n][:], 0.5)
                nc.scalar.dma_start(
                    out[n][orow : orow + nrows // 2, :], ob[n][:]
                )

    nc.finalize()
    return nc


def _run(x, trace=False):
    from concourse.bass_utils import run_bass_kernel_spmd

    if "nc" not in _cache:
        _cache["nc"] = _build_program()
    nc = _cache["nc"]

    x = np.ascontiguousarray(np.asarray(x), dtype=np.float32)
    in_maps = [{"x": x[i].reshape(ROWS, W)} for i in range(N_CORES)]
    res = run_bass_kernel_spmd(nc, in_maps, core_ids=list(range(N_CORES)), trace=trace)
    _cache["last_results"] = res

    outs = []
    for n in ("ll", "lh", "hl", "hh"):
        outs.append(
            np.stack([res.results[i][n].reshape(C, H // 2, W // 2)
                      for i in range(N_CORES)])
        )
    return tuple(outs)


def kernel(x):
    return _run(x, trace=False)


# revision 4
# speedup vs baseline: 1.2731x; 1.0122x over previous
"""2D Haar DWT (periodization) on Trainium2, data-parallel over 8 NeuronCores.

Memory-bound problem; the floor is the 16 SDMA engines at ~27 GB/s each
(descriptor cost ~6.6 ns + 37.5 ns/KiB).  Design:
  - batch -> 8 cores; per core [32,512,512] = 16384 rows of 512 f32,
    tiled 8 rows/partition (2 MiB input tiles, contiguous >=4 KiB lines).
  - stage 1 (H butterfly) on DVE at full rate; stage 2 (W butterfly,
    inherently stride-2, 1/4-rate on DVE) split ll/hl on DVE + lh/hh on
    GpSimd so no engine exceeds the 9.8 us/tile DMA-queue work.
  - single x0.5 on ScalarE per subband (exact: power of two).
  - Sync sequencer triggers ONLY input DMAs (prefetch never blocks on
    store semaphores); ScalarE triggers each store right after its own
    mul, same-engine program order.  gpsimd.dma_start is SWDGE -- never.
  - front taper (4x smaller first tiles) shortens the ramp; tail taper
    uses PAIR-MERGED stores (ll|hl, lh|hh regions of one DRAM tensor,
    2 muls + 2 triggers per tile) because the drain is gated by ScalarE's
    serial mul+trigger chains (~590 ns per DMA trigger).

Steady state: SDMA engines 95-97% busy; ~181-188 us vs ~212-219 us
baseline and a ~178 us descriptor-rate floor.

Input  x: [8, 32, 512, 512] f32 -> batch -> 8 cores.
Output (LL, LH, HL, HH), each [8, 32, 256, 256] f32.
"""

import sys

import numpy as np

if "/opt/trn_rl_repo" not in sys.path:
    sys.path.insert(0, "/opt/trn_rl_repo")

B, C, H, W = 8, 32, 512, 512
ROWS = C * H              # 16384 flat rows per core
RPP = 8                   # input rows per partition (must be even)
TILE_ROWS = 128 * RPP     # 1024
OROWS = ROWS // 2         # 8192 output rows per subband per core
N_CORES = 8

TAPER = 4
QT = TILE_ROWS // TAPER   # 256
PLAN = [(k * QT, QT) for k in range(TAPER)]                   # front taper
PLAN += [(i * TILE_ROWS, TILE_ROWS)
         for i in range(1, ROWS // TILE_ROWS - 1)]            # full tiles
PLAN += [((ROWS - TILE_ROWS) + k * QT, QT) for k in range(TAPER)]  # tail taper

SUBBANDS = ("ll", "lh", "hl", "hh")

_cache = {}


def _build_program():
    from concourse import bacc, mybir
    from concourse.tile import TileContext

    f32 = mybir.dt.float32
    add = mybir.AluOpType.add
    sub = mybir.AluOpType.subtract

    nc = bacc.Bacc()
    x = nc.dram_tensor("x", [ROWS, W], f32, kind="ExternalInput")
    o = nc.dram_tensor("o", [4 * OROWS, W // 2], f32, kind="ExternalOutput")
    # region order: ll, hl (DVE pair) | lh, hh (GpSimd pair)
    o4 = o.rearrange("(s r) w -> s r w", s=4)

    with TileContext(nc) as tc, tc.tile_pool(name="p", bufs=4) as pool:
        for r0, nrows in PLAN:
            rpp = nrows // 128        # rows per partition this tile
            jp = rpp // 2             # H-pairs per partition
            tin = pool.tile([128, rpp * W], f32, tag="tin",
                            padded_shape=[128, RPP * W])
            nc.sync.dma_start(tin[:], x[r0 : r0 + nrows, :])

            t4 = tin.rearrange("p (j o w) -> p j o w", j=jp, o=2)
            e = t4[:, :, 0, :]    # even H rows  [128, jp, 512]
            o = t4[:, :, 1, :]    # odd H rows   [128, jp, 512]

            s = pool.tile([128, jp * W], f32, tag="s",
                          padded_shape=[128, (RPP // 2) * W])
            d = pool.tile([128, jp * W], f32, tag="d",
                          padded_shape=[128, (RPP // 2) * W])
            s3 = s.rearrange("p (j w) -> p j w", j=jp)
            d3 = d.rearrange("p (j w) -> p j w", j=jp)
            nc.vector.tensor_add(out=s3, in0=e, in1=o)
            nc.vector.tensor_sub(out=d3, in0=e, in1=o)

            s4 = s.rearrange("p (j k o) -> p j k o", j=jp, o=2)
            d4 = d.rearrange("p (j k o) -> p j k o", j=jp, o=2)
            se, so = s4[:, :, :, 0], s4[:, :, :, 1]
            de, do = d4[:, :, :, 0], d4[:, :, :, 1]

            K = W // 2
            orow = r0 // 2
            h = nrows // 2
            tail = (r0 >= ROWS - TILE_ROWS) and nrows != TILE_ROWS
            if not tail:
                # v4 path: separate ob tile + store per subband.
                # o4 region order is (ll, hl, lh, hh).
                obs = ("ll", "hl", "lh", "hh")
                ob = {n: pool.tile([128, jp * K], f32, tag=n, name=n,
                                   padded_shape=[128, (RPP // 2) * K])
                      for n in obs}
                nc.vector.tensor_tensor(out=ob["ll"][:], in0=se, in1=so, op=add)
                nc.gpsimd.tensor_tensor(out=ob["lh"][:], in0=de, in1=do, op=add)
                nc.vector.tensor_tensor(out=ob["hl"][:], in0=se, in1=so, op=sub)
                nc.gpsimd.tensor_tensor(out=ob["hh"][:], in0=de, in1=do, op=sub)
                for i, n in enumerate(obs):
                    nc.scalar.mul(ob[n][:], ob[n][:], 0.5)
                    nc.scalar.dma_start(o4[i, orow : orow + h, :], ob[n][:])
            else:
                # tail taper: pair-merged stores halve the serial ScalarE
                # mul+trigger chains that dominate the pipeline drain.
                # 2*jp*K (taper) <= (RPP//2)*K, so reuse the ll/lh rings.
                obA = pool.tile([128, 2 * jp * K], f32, tag="ll",
                                padded_shape=[128, (RPP // 2) * K])
                obB = pool.tile([128, 2 * jp * K], f32, tag="lh",
                                padded_shape=[128, (RPP // 2) * K])
                vA = obA.rearrange("p (s j w) -> p s j w", s=2, j=jp)
                vB = obB.rearrange("p (s j w) -> p s j w", s=2, j=jp)
                nc.vector.tensor_tensor(out=vA[:, 0], in0=se, in1=so, op=add)
                nc.gpsimd.tensor_tensor(out=vB[:, 0], in0=de, in1=do, op=add)
                nc.vector.tensor_tensor(out=vA[:, 1], in0=se, in1=so, op=sub)
                nc.gpsimd.tensor_tensor(out=vB[:, 1], in0=de, in1=do, op=sub)
                for base, ob_, vv in ((0, obA, vA), (2, obB, vB)):
                    nc.scalar.mul(ob_[:], ob_[:], 0.5)
                    dv = o4[base : base + 2, orow : orow + h, :].rearrange(
                        "s (p j) w -> p s j w", j=jp
                    )
                    nc.scalar.dma_start(dv, vv)

    nc.finalize()
    return nc


def _run(x, trace=False):
    from concourse.bass_utils import run_bass_kernel_spmd

    if "nc" not in _cache:
        _cache["nc"] = _build_program()
    nc = _cache["nc"]

    x = np.ascontiguousarray(np.asarray(x), dtype=np.float32)
    in_maps = [{"x": x[i].reshape(ROWS, W)} for i in range(N_CORES)]
    res = run_bass_kernel_spmd(nc, in_maps, core_ids=list(range(N_CORES)), trace=trace)
    _cache["last_results"] = res

    full = np.stack([res.results[i]["o"].reshape(4, C, H // 2, W // 2)
                     for i in range(N_CORES)])   # regions: ll, hl, lh, hh
    return (full[:, 0], full[:, 2], full[:, 1], full[:, 3])


def kernel(x):
    return _run(x, trace=False)
